# revision 20
# baseline (speedup 1.0000x reference)
"""DirGATConv on 8 Trainium2 NeuronCores (Bass/Tile).

Problem: nn_DirGATConv  (N=50000 nodes, E=800000 edges, DIN=128, DOUT=64)
    out = 0.5 * GATConv(x, src->dst, W1) + 0.5 * GATConv(x, dst->src, W2)

Strategy (zero collectives):
  * Nodes are grouped into 128-row "chunks"; chunks are assigned to cores in
    contiguous ranges (49 chunk slots per core).  Conv1 groups edges by dst,
    conv2 by src; both groupings use the same node chunks, so each core owns
    all edges whose *group* node lies in its range and produces the final
    output rows for exactly its node range.  The host concatenates slices.
  * Phase A (replicated on every core): one fused matmul computes
    xw1|xw2|attention projections for all nodes; rows of the per-conv gather
    tables T1/T2 are [xw (64 f16) | a_src (f16) | ...] = 256 B, plus flat
    f32 a_dst arrays.
  * Phase B: per-edge source rows are fetched with the custom DMA gather
    (InstDMAGatherAnt, int16 indices, so the node space is split into two
    banks with separate slot spaces).  Per-edge dst-attention is expanded
    from a per-chunk broadcast tile through the one-hot matrix
    (mul + reduce).  Messages [w*xw | w] are accumulated per 128-node chunk
    on the PE via one-hot matmuls into PSUM [128, 65]; the denominator is
    column 64.  Softmax max-subtraction is skipped (|scores| <= ~3, exp is
    safe in fp32, and the quotient is unchanged).
"""

import math

import numpy as np

import concourse.bass as bass
import concourse.mybir as mybir
import concourse.tile as tile
from concourse import bacc, bass_utils
from concourse._compat import with_exitstack

# ---------------------------------------------------------------- constants
N = 50000
E = 800000
DIN = 128
DOUT = 64
ALPHA = 0.5
NEG_SLOPE = 0.2
NCORES = 8
P = 128

G_REAL = math.ceil(N / P)              # 391 real node chunks
CPC = math.ceil(G_REAL / NCORES)       # 49 chunk slots per core
G = NCORES * CPC                       # 392 padded chunk slots
NT = G * P                             # 50176 padded node count
NPC = CPC * P                          # 6272 nodes per core
BANK = 32768                           # int16 gather-index bank size
RND = 2                                # node tiles per PSUM round in phase A
ADG = 4                                # node tiles per a_d matmul group
SCN = 3                                # chunks per gather super-chunk
GMAX = 8                               # max 128-blocks per gather (1024-desc carveout)
SCOL = (64, 66)                        # a_src column in T1 / T2 rows

f32 = mybir.dt.float32
f16 = mybir.dt.float16
i16 = mybir.dt.int16

_CACHE = {}


# ------------------------------------------------------------ host preprocess
def _edge_arrays(key, gidx):
    """Bin edges by group node ("key"), 128 nodes per chunk; within a chunk
    split edges by gather-node bank (<BANK or >=BANK) into two slot spaces.
    Slot i of a span maps to partition i%128, block i//128.  Returns per-bank
    (idx16 wrapped [NCORES,128,Ktot*8], dl [NCORES,128,Ktot]) plus per-bank
    block counts kbu[CPC] (shared across cores)."""
    order = np.lexsort((key, gidx >= BANK, key // P))
    key_s = key[order].astype(np.int64)
    gid_s = gidx[order].astype(np.int64)
    bank_s = (gid_s >= BANK).astype(np.int64)

    chunk = key_s // P
    # counts per (chunk, bank)
    cb = chunk * 2 + bank_s
    counts = np.bincount(cb, minlength=2 * G).reshape(G, 2)
    kb = -(-counts // P)                                  # [G, 2]
    kbA = np.maximum(kb[:, 0].reshape(NCORES, CPC).max(axis=0), 1)
    kbB = kb[:, 1].reshape(NCORES, CPC).max(axis=0)
    out = []
    start = np.zeros(2 * G + 1, np.int64)
    start[1:] = np.cumsum(counts.reshape(-1))
    within = np.arange(key_s.size, dtype=np.int64) - start[cb]
    core = chunk // CPC
    slot = chunk % CPC
    for b, kbu in ((0, kbA), (1, kbB)):
        BO = np.zeros(CPC + 1, np.int64)
        BO[1:] = np.cumsum(kbu)
        ktot = int(BO[-1])
        sel = bank_s == b
        s = BO[slot[sel]] * P + within[sel]
        idx = np.zeros((NCORES, max(ktot, 1) * P), np.int16)
        dl = np.full((NCORES, max(ktot, 1) * P), -1.0, np.float16)
        idx[core[sel], s] = (gid_s[sel] - b * BANK).astype(np.int16)
        dl[core[sel], s] = (key_s[sel] - chunk[sel] * P).astype(np.float16)
        # wrap int16 indices: position i -> [p=i%16, j=i//16], replicated x8
        idxw = idx.reshape(NCORES, max(ktot, 1) * 8, 16).transpose(0, 2, 1)
        idxw = np.ascontiguousarray(np.tile(idxw, (1, 8, 1)))
        dl2 = np.ascontiguousarray(
            dl.reshape(NCORES, max(ktot, 1), P).transpose(0, 2, 1))
        out.append((idxw, dl2, tuple(int(x) for x in kbu)))
    return out


def _preprocess(x, edge_index, W1, att_src1, att_dst1, b1,
                W2, att_src2, att_dst2, b2):
    src = np.asarray(edge_index[0], np.int64)
    dst = np.asarray(edge_index[1], np.int64)
    loops = np.arange(N, dtype=np.int64)
    all_src = np.concatenate([src, loops])
    all_dst = np.concatenate([dst, loops])

    c1 = _edge_arrays(all_dst, all_src)    # conv1: group by dst, gather src
    c2 = _edge_arrays(all_src, all_dst)    # conv2: group by src, gather dst

    xT = np.zeros((DIN, NT), np.float16)
    xT[:, :N] = np.asarray(x, np.float32).T.astype(np.float16)

    wfull = np.zeros((DIN, 192), np.float32)
    wfull[:, 0:64] = W1
    wfull[:, 64:128] = W2
    wfull[:, 128] = W1 @ att_src1
    wfull[:, 129] = W1 @ att_dst1
    wfull[:, 130] = W2 @ att_src2
    wfull[:, 131] = W2 @ att_dst2
    wfull = wfull.astype(np.float16)
    adw = np.stack([W1 @ att_dst1, W2 @ att_dst2], axis=1).astype(np.float16)

    iota = np.broadcast_to(np.arange(P, dtype=np.float16), (P, P)).copy()
    bcomb = np.broadcast_to(
        ((1.0 - ALPHA) * b1 + ALPHA * b2).astype(np.float32), (P, DOUT)).copy()

    common = dict(xT=xT, wfull=wfull, adw=adw, iota=iota, bcomb=bcomb)
    per_core = []
    for k in range(NCORES):
        d = {}
        for cv, banks in (("1", c1), ("2", c2)):
            for bn, (idxw, dl2, _kbu) in zip("ab", banks):
                d["ix" + cv + bn] = idxw[k]
                d["dl" + cv + bn] = dl2[k]
        # per-chunk a_d row ids (global chunk index), wrapped for dma_gather
        af = np.repeat(np.arange(k * CPC, (k + 1) * CPC), P).astype(np.int16)
        d["adix"] = np.tile(af.reshape(CPC * 8, 16).T, (8, 1)).astype(np.int16)
        per_core.append(d)
    kbus = tuple(banks[b][2] for banks in (c1, c2) for b in (0, 1))
    return common, per_core, kbus


# ------------------------------------------------------------- device program
@with_exitstack
def _emit(ctx, tc, outs, ins, kbus):
    nc = tc.nc
    out_d = outs["out"]
    kbu1a, kbu1b, kbu2a, kbu2b = kbus

    t1_d = nc.dram_tensor("T1_tab", [NT, P], f16, kind="Internal").ap()
    t2_d = nc.dram_tensor("T2_tab", [NT, P], f16, kind="Internal").ap()
    ad1_d = nc.dram_tensor("ad1", [NT, 1], f32, kind="Internal").ap()
    ad2_d = nc.dram_tensor("ad2", [NT, 1], f32, kind="Internal").ap()

    # ---------------- phase A: gather tables + a_d arrays (replicated) -------
    with tc.tile_pool(name="pa", bufs=2) as pa, \
         tc.tile_pool(name="pa1", bufs=1) as pa1, \
         tc.tile_pool(name="pap", bufs=3, space="PSUM") as pap, \
         tc.tile_pool(name="pad", bufs=2, space="PSUM") as pad:
        wf = pa1.tile([P, 192], f16)
        nc.sync.dma_start(out=wf[:], in_=ins["wfull"][:])
        adw = pa1.tile([P, 2], f16)
        nc.sync.dma_start(out=adw[:], in_=ins["adw"][:])
        t_views = [t.rearrange("(g p) c -> p g c", p=P) for t in (t1_d, t2_d)]
        ad_views = [a.rearrange("(g p) c -> g (p c)", p=P) for a in (ad1_d, ad2_d)]
        for piece in range(NCORES):
            xt = pa.tile([P, NPC], f16, tag="xt")
            nc.sync.dma_start(
                out=xt[:], in_=ins["xT"][:, piece * NPC:(piece + 1) * NPC])
            # a_d arrays: [2, 512] psum per 4-tile group, stored straight out
            for j0 in range(0, CPC, ADG):
                r = min(ADG, CPC - j0)
                g0 = piece * CPC + j0
                psa = pad.tile([2, ADG * P], f32, tag="pad")
                nc.tensor.matmul(out=psa[:, :r * P], lhsT=adw[:],
                                 rhs=xt[:, j0 * P:(j0 + r) * P],
                                 start=True, stop=True)
                adsb = pa.tile([2, ADG * P], f32, tag="adsb")
                nc.vector.tensor_copy(out=adsb[:, :r * P], in_=psa[:, :r * P])
                for ti, adv in enumerate(ad_views):
                    nc.sync.dma_start(
                        out=adv[g0:g0 + r, :].rearrange("g c -> (g c)")[None, :],
                        in_=adsb[ti:ti + 1, :r * P])
            # gather tables
            for j0 in range(0, CPC, RND):
                r = min(RND, CPC - j0)
                g0 = piece * CPC + j0
                ps = pap.tile([P, 192 * RND], f32, tag="pap")
                for q in range(r):
                    nc.tensor.matmul(
                        out=ps[:, 192 * q:192 * (q + 1)],
                        lhsT=xt[:, (j0 + q) * P:(j0 + q + 1) * P],
                        rhs=wf[:], start=True, stop=True)
                psv = ps[:].rearrange("p (q c) -> p q c", q=RND)
                for ti, xw0 in ((0, 0), (1, 64)):
                    trow = pa.tile([P, RND * P], f16, tag=f"t{ti}r")
                    tv = trow[:].rearrange("p (q c) -> p q c", q=RND)
                    nc.vector.tensor_copy(
                        out=tv[:, :r, :], in_=psv[:, :r, xw0:xw0 + P])
                    if ti == 0:
                        nc.vector.tensor_copy(
                            out=tv[:, :r, 64:66], in_=psv[:, :r, 128:130])
                    nc.sync.dma_start(
                        out=t_views[ti][:, g0:g0 + r, :], in_=tv[:, :r, :])

    # ---------------- phase B: edge aggregation ----------------
    with tc.tile_pool(name="pre", bufs=1) as pre, \
         tc.tile_pool(name="pb", bufs=3) as pb, \
         tc.tile_pool(name="pg", bufs=2) as pg, \
         tc.tile_pool(name="pbp", bufs=4, space="PSUM") as pbp:
        iota = pre.tile([P, P], f16)
        nc.sync.dma_start(out=iota[:], in_=ins["iota"][:])
        bcomb = pre.tile([P, DOUT], f32)
        nc.sync.dma_start(out=bcomb[:], in_=ins["bcomb"][:])

        spaces = []   # (cv, bank, kbu, bo, maxw, table_ap, scol)
        for cv, tab, scol, kbua, kbub in (
                ("1", t1_d, SCOL[0], kbu1a, kbu1b),
                ("2", t2_d, SCOL[1], kbu2a, kbu2b)):
            for bn, kbu in (("a", kbua), ("b", kbub)):
                bo = np.zeros(CPC + 1, np.int64)
                bo[1:] = np.cumsum(kbu)
                maxw = max(int(bo[min(s + SCN, CPC)] - bo[s])
                           for s in range(0, CPC, SCN))
                tap = tab if (bn == "a" or NT <= BANK) else tab[BANK:, :]
                spaces.append(dict(cv=cv, bn=bn, kbu=kbu, bo=bo, maxw=maxw,
                                   tab=tap, scol=scol))
        kbmax = max(max(sp["kbu"]) for sp in spaces)
        dls = {}
        for sp in spaces:
            keyn = sp["cv"] + sp["bn"]
            kt = max(sum(sp["kbu"]), 1)
            t = pre.tile([P, kt], f16, tag="dl" + keyn)
            nc.sync.dma_start(out=t[:], in_=ins["dl" + keyn][:])
            dls[keyn] = t

        ad_tabs = {"1": ad1_d.rearrange("(g p) c -> g (p c)", p=P),
                   "2": ad2_d.rearrange("(g p) c -> g (p c)", p=P)}
        adix = pre.tile([P, CPC * 8], i16)
        nc.sync.dma_start(out=adix[:], in_=ins["adix"][:])

        for sc in range(0, CPC, SCN):
            scr = min(SCN, CPC - sc)
            sc_g = {}
            adbcs = {}
            for cv in ("1", "2"):
                t = pg.tile([P, SCN, P], f32, tag="adbc" + cv)
                nc.gpsimd.dma_gather(
                    out_ap=t[:, :scr, :], in_ap=ad_tabs[cv],
                    idxs_ap=adix[:, sc * 8:(sc + scr) * 8],
                    num_idxs=scr * P, num_idxs_reg=scr * P, elem_size=P)
                adbcs[cv] = t
            for sp in spaces:
                j0, j1 = int(sp["bo"][sc]), int(sp["bo"][sc + scr])
                jw = j1 - j0
                keyn = sp["cv"] + sp["bn"]
                if jw == 0:
                    sc_g[keyn] = (None, j0)
                    continue
                ixt = pg.tile([P, max(sp["maxw"], 1) * 8], i16, tag="ix" + keyn)
                nc.sync.dma_start(
                    out=ixt[:, :jw * 8],
                    in_=ins["ix" + keyn][:, j0 * 8:j1 * 8])
                gt = pg.tile([P, max(sp["maxw"], 1), P], f16, tag="gt" + keyn)
                for g0 in range(0, jw, GMAX):
                    gw = min(GMAX, jw - g0)
                    nc.gpsimd.dma_gather(
                        out_ap=gt[:, g0:g0 + gw, :], in_ap=sp["tab"],
                        idxs_ap=ixt[:, g0 * 8:(g0 + gw) * 8], num_idxs=gw * P,
                        num_idxs_reg=gw * P, elem_size=P)
                sc_g[keyn] = (gt, j0)

            for c in range(sc, sc + scr):
                o1 = None
                psums = {}
                for cv in ("1", "2"):
                    ps = pbp.tile([P, 65], f32, tag="ps" + cv)
                    psums[cv] = ps
                    started = False
                    cvsp = [sp for sp in spaces if sp["cv"] == cv]
                    last_sp = [sp for sp in cvsp if int(sp["kbu"][c]) > 0][-1]
                    for sp in cvsp:
                        kb = int(sp["kbu"][c])
                        if kb == 0:
                            continue
                        keyn = cv + sp["bn"]
                        gt, j0 = sc_g[keyn]
                        cj = int(sp["bo"][c]) - j0
                        gts = gt[:, cj:cj + kb, :]
                        # one-hot
                        oh = pb.tile([P, kbmax, P], f16, tag="oh")
                        dlv = dls[keyn][:, int(sp["bo"][c]):
                                        int(sp["bo"][c]) + kb]
                        i1 = iota[:].unsqueeze(1)
                        nc.vector.tensor_tensor(
                            out=oh[:, :kb, :],
                            in0=dlv.unsqueeze(-1).to_broadcast([P, kb, P]),
                            in1=bass.AP(i1.tensor, i1.offset,
                                        [i1.ap[0], [0, kb], i1.ap[2]]),
                            op=mybir.AluOpType.is_equal)
                        # expand a_d through the one-hot: mul + reduce
                        ohw = pb.tile([P, kbmax, P], f16, tag="ohw")
                        ab = adbcs[cv][:, c - sc, :].unsqueeze(1)
                        nc.vector.tensor_tensor(
                            out=ohw[:, :kb, :], in0=oh[:, :kb, :],
                            in1=bass.AP(ab.tensor, ab.offset,
                                        [ab.ap[0], [0, kb], ab.ap[2]]),
                            op=mybir.AluOpType.mult)
                        ade = pb.tile([P, kbmax], f32, tag="ade")
                        nc.vector.tensor_reduce(
                            out=ade[:, :kb], in_=ohw[:, :kb, :],
                            axis=mybir.AxisListType.X, op=mybir.AluOpType.add)
                        # scores
                        u = pb.tile([P, kbmax], f32, tag="u")
                        nc.vector.tensor_tensor(
                            out=u[:, :kb], in0=gts[:, :, sp["scol"]],
                            in1=ade[:, :kb], op=mybir.AluOpType.add)
                        u2 = pb.tile([P, kbmax], f32, tag="u2")
                        nc.vector.tensor_scalar_mul(
                            u2[:, :kb], u[:, :kb], NEG_SLOPE)
                        nc.vector.tensor_tensor(
                            out=u2[:, :kb], in0=u[:, :kb], in1=u2[:, :kb],
                            op=mybir.AluOpType.max)
                        w = pb.tile([P, kbmax], f16, tag="w")
                        nc.scalar.activation(
                            out=w[:, :kb], in_=u2[:, :kb],
                            func=mybir.ActivationFunctionType.Exp)
                        # messages [w*xw | w]
                        msg = pb.tile([P, kbmax, 65], f16, tag="msg")
                        nc.vector.tensor_tensor(
                            out=msg[:, :kb, 0:64], in0=gts[:, :, 0:64],
                            in1=w[:, :kb].unsqueeze(-1).to_broadcast(
                                [P, kb, 64]),
                            op=mybir.AluOpType.mult)
                        nc.vector.tensor_copy(
                            out=msg[:, :kb, 64], in_=w[:, :kb])
                        for j in range(kb):
                            nc.tensor.matmul(
                                out=ps[:], lhsT=oh[:, j, :], rhs=msg[:, j, :],
                                start=not started,
                                stop=(sp is last_sp and j == kb - 1))
                            started = True
                # finalize
                for cv in ("1", "2"):
                    ps = psums[cv]
                    den = pb.tile([P, 1], f32, tag="den")
                    nc.vector.tensor_scalar_max(den[:], ps[:, 64:65], 1e-30)
                    rec = pb.tile([P, 1], f32, tag="rec")
                    nc.vector.reciprocal(out=rec[:], in_=den[:])
                    rec2 = pb.tile([P, 1], f32, tag="rec2")
                    nc.vector.tensor_scalar_mul(
                        rec2[:], rec[:], (1.0 - ALPHA) if cv == "1" else ALPHA)
                    o = pb.tile([P, DOUT], f32, tag="o" + cv)
                    nc.vector.tensor_scalar(
                        out=o[:], in0=ps[:, 0:64], scalar1=rec2[:],
                        scalar2=None, op0=mybir.AluOpType.mult)
                    if cv == "1":
                        o1 = o
                ofin = pb.tile([P, DOUT], f32, tag="ofin")
                nc.vector.tensor_tensor(
                    out=ofin[:], in0=o1[:], in1=o[:], op=mybir.AluOpType.add)
                nc.vector.tensor_tensor(
                    out=ofin[:], in0=ofin[:], in1=bcomb[:],
                    op=mybir.AluOpType.add)
                nc.sync.dma_start(out=out_d[c * P:(c + 1) * P, :], in_=ofin[:])


def _build(kbus):
    nc = bacc.Bacc("TRN2", target_bir_lowering=False, debug=False,
                   num_devices=NCORES)
    kbu1a, kbu1b, kbu2a, kbu2b = kbus
    ins = {
        "xT": nc.dram_tensor("xT", [DIN, NT], f16, kind="ExternalInput").ap(),
        "adix": nc.dram_tensor("adix", [P, CPC * 8], i16,
                               kind="ExternalInput").ap(),
        "wfull": nc.dram_tensor("wfull", [DIN, 192], f16,
                                kind="ExternalInput").ap(),
        "adw": nc.dram_tensor("adw", [DIN, 2], f16,
                              kind="ExternalInput").ap(),
        "iota": nc.dram_tensor("iota", [P, P], f16, kind="ExternalInput").ap(),
        "bcomb": nc.dram_tensor("bcomb", [P, DOUT], f32,
                                kind="ExternalInput").ap(),
    }
    for nm, kbu in (("1a", kbu1a), ("1b", kbu1b), ("2a", kbu2a),
                    ("2b", kbu2b)):
        kt = max(sum(kbu), 1)
        ins["ix" + nm] = nc.dram_tensor(
            "ix" + nm, [P, kt * 8], i16, kind="ExternalInput").ap()
        ins["dl" + nm] = nc.dram_tensor(
            "dl" + nm, [P, kt], f16, kind="ExternalInput").ap()
    outs = {"out": nc.dram_tensor("out", [NPC, DOUT], f32,
                                  kind="ExternalOutput").ap()}
    with tile.TileContext(nc) as tc:
        _emit(tc, outs, ins, kbus)
    nc.compile()
    return nc


# ------------------------------------------------------------------- entry
def kernel(x, edge_index, W1, att_src1, att_dst1, b1,
           W2, att_src2, att_dst2, b2):
    common, per_core, kbus = _preprocess(
        np.asarray(x), np.asarray(edge_index),
        np.asarray(W1, np.float64), np.asarray(att_src1, np.float64),
        np.asarray(att_dst1, np.float64), np.asarray(b1, np.float32),
        np.asarray(W2, np.float64), np.asarray(att_src2, np.float64),
        np.asarray(att_dst2, np.float64), np.asarray(b2, np.float32))

    if kbus not in _CACHE:
        _CACHE[kbus] = _build(kbus)
    nc = _CACHE[kbus]

    in_maps = [dict(common, **pc) for pc in per_core]
    res = bass_utils.run_bass_kernel_spmd(
        nc, in_maps, core_ids=list(range(NCORES)))
    full = np.concatenate(
        [res.results[k]["out"] for k in range(NCORES)], axis=0)
    return np.ascontiguousarray(full[:N]).astype(np.float32)


# revision 23
# speedup vs baseline: 1.2306x; 1.2306x over previous
"""DirGATConv on 8 Trainium2 NeuronCores (Bass/Tile).

Problem: nn_DirGATConv  (N=50000 nodes, E=800000 edges, DIN=128, DOUT=64)
    out = 0.5 * GATConv(x, src->dst, W1) + 0.5 * GATConv(x, dst->src, W2)

Strategy (zero collectives):
  * Nodes are grouped into 128-row "chunks"; chunks are assigned to cores in
    contiguous ranges (49 chunk slots per core).  Conv1 groups edges by dst,
    conv2 by src; both groupings use the same node chunks, so each core owns
    all edges whose *group* node lies in its range and produces the final
    output rows for exactly its node range.  The host concatenates slices.
  * Phase A (replicated on every core): one fused matmul computes
    xw1|xw2|attention projections for all nodes; rows of the per-conv gather
    tables T1/T2 are [xw (64 f16) | a_src (f16) | ...] = 256 B, plus flat
    f32 a_dst arrays.
  * Phase B: per-edge source rows are fetched with the custom DMA gather
    (InstDMAGatherAnt, int16 indices, so the node space is split into two
    banks with separate slot spaces).  Per-edge dst-attention is expanded
    from a per-chunk broadcast tile through the one-hot matrix
    (mul + reduce).  Messages [w*xw | w] are accumulated per 128-node chunk
    on the PE via one-hot matmuls into PSUM [128, 65]; the denominator is
    column 64.  Softmax max-subtraction is skipped (|scores| <= ~3, exp is
    safe in fp32, and the quotient is unchanged).
"""

import math

import numpy as np

import concourse.bass as bass
import concourse.mybir as mybir
import concourse.tile as tile
from concourse import bacc, bass_utils
from concourse._compat import with_exitstack

# ---------------------------------------------------------------- constants
N = 50000
E = 800000
DIN = 128
DOUT = 64
ALPHA = 0.5
NEG_SLOPE = 0.2
NCORES = 8
P = 128

G_REAL = math.ceil(N / P)              # 391 real node chunks
CPC = math.ceil(G_REAL / NCORES)       # 49 chunk slots per core
G = NCORES * CPC                       # 392 padded chunk slots
NT = G * P                             # 50176 padded node count
NPC = CPC * P                          # 6272 nodes per core
BANK = 32768                           # int16 gather-index bank size
RND = 2                                # node tiles per PSUM round in phase A
ADG = 4                                # node tiles per a_d matmul group
SCN = 3                                # chunks per gather super-chunk
GMAX = 8                               # max 128-blocks per gather (1024-desc carveout)
SCOL = (64, 66)                        # a_src column in T1 / T2 rows

f32 = mybir.dt.float32
f16 = mybir.dt.float16
i16 = mybir.dt.int16

VARIANT = "full"    # "full" | "gathers" | "phasea"  (perf-bisect variants)

_CACHE = {}


# ------------------------------------------------------------ host preprocess
def _edge_arrays(key, gidx):
    """Bin edges by group node ("key"), 128 nodes per chunk; within a chunk
    split edges by gather-node bank (<BANK or >=BANK) into two slot spaces.
    Slot i of a span maps to partition i%128, block i//128.  Returns per-bank
    (idx16 wrapped [NCORES,128,Ktot*8], dl [NCORES,128,Ktot]) plus per-bank
    block counts kbu[CPC] (shared across cores)."""
    order = np.lexsort((key, gidx >= BANK, key // P))
    key_s = key[order].astype(np.int64)
    gid_s = gidx[order].astype(np.int64)
    bank_s = (gid_s >= BANK).astype(np.int64)

    chunk = key_s // P
    # counts per (chunk, bank)
    cb = chunk * 2 + bank_s
    counts = np.bincount(cb, minlength=2 * G).reshape(G, 2)
    kb = -(-counts // P)                                  # [G, 2]
    kbA = np.maximum(kb[:, 0].reshape(NCORES, CPC).max(axis=0), 1)
    kbB = kb[:, 1].reshape(NCORES, CPC).max(axis=0)
    out = []
    start = np.zeros(2 * G + 1, np.int64)
    start[1:] = np.cumsum(counts.reshape(-1))
    within = np.arange(key_s.size, dtype=np.int64) - start[cb]
    core = chunk // CPC
    slot = chunk % CPC
    for b, kbu in ((0, kbA), (1, kbB)):
        BO = np.zeros(CPC + 1, np.int64)
        BO[1:] = np.cumsum(kbu)
        ktot = int(BO[-1])
        sel = bank_s == b
        s = BO[slot[sel]] * P + within[sel]
        idx = np.zeros((NCORES, max(ktot, 1) * P), np.int16)
        dl = np.full((NCORES, max(ktot, 1) * P), -1.0, np.float16)
        idx[core[sel], s] = (gid_s[sel] - b * BANK).astype(np.int16)
        dl[core[sel], s] = (key_s[sel] - chunk[sel] * P).astype(np.float16)
        # wrap int16 indices: position i -> [p=i%16, j=i//16], replicated x8
        idxw = idx.reshape(NCORES, max(ktot, 1) * 8, 16).transpose(0, 2, 1)
        idxw = np.ascontiguousarray(np.tile(idxw, (1, 8, 1)))
        dl2 = np.ascontiguousarray(
            dl.reshape(NCORES, max(ktot, 1), P).transpose(0, 2, 1))
        out.append((idxw, dl2, tuple(int(x) for x in kbu)))
    return out


def _preprocess(x, edge_index, W1, att_src1, att_dst1, b1,
                W2, att_src2, att_dst2, b2):
    src = np.asarray(edge_index[0], np.int64)
    dst = np.asarray(edge_index[1], np.int64)
    loops = np.arange(N, dtype=np.int64)
    all_src = np.concatenate([src, loops])
    all_dst = np.concatenate([dst, loops])

    c1 = _edge_arrays(all_dst, all_src)    # conv1: group by dst, gather src
    c2 = _edge_arrays(all_src, all_dst)    # conv2: group by src, gather dst

    xT = np.zeros((DIN, NT), np.float16)
    xT[:, :N] = np.asarray(x, np.float32).T.astype(np.float16)

    wfull = np.zeros((DIN, 192), np.float32)
    wfull[:, 0:64] = W1
    wfull[:, 64:128] = W2
    wfull[:, 128] = W1 @ att_src1
    wfull[:, 129] = W1 @ att_dst1
    wfull[:, 130] = W2 @ att_src2
    wfull[:, 131] = W2 @ att_dst2
    wfull = wfull.astype(np.float16)
    adw = np.stack([W1 @ att_dst1, W2 @ att_dst2], axis=1).astype(np.float16)

    iota = np.broadcast_to(np.arange(P, dtype=np.float16), (P, P)).copy()
    bcomb = np.broadcast_to(
        ((1.0 - ALPHA) * b1 + ALPHA * b2).astype(np.float32), (P, DOUT)).copy()

    common = dict(xT=xT, wfull=wfull, adw=adw, iota=iota, bcomb=bcomb)
    per_core = []
    for k in range(NCORES):
        d = {}
        for cv, banks in (("1", c1), ("2", c2)):
            for bn, (idxw, dl2, _kbu) in zip("ab", banks):
                d["ix" + cv + bn] = idxw[k]
                d["dl" + cv + bn] = dl2[k]
        # per-chunk a_d row ids (global chunk index), wrapped for dma_gather
        af = np.repeat(np.arange(k * CPC, (k + 1) * CPC), P).astype(np.int16)
        d["adix"] = np.tile(af.reshape(CPC * 8, 16).T, (8, 1)).astype(np.int16)
        per_core.append(d)
    kbus = tuple(banks[b][2] for banks in (c1, c2) for b in (0, 1))
    return common, per_core, kbus


# ------------------------------------------------------------- device program
@with_exitstack
def _emit(ctx, tc, outs, ins, kbus):
    nc = tc.nc
    out_d = outs["out"]
    kbu1a, kbu1b, kbu2a, kbu2b = kbus

    t1_d = nc.dram_tensor("T1_tab", [NT, P], f16, kind="Internal").ap()
    t2_d = nc.dram_tensor("T2_tab", [NT, P], f16, kind="Internal").ap()
    ad1_d = nc.dram_tensor("ad1", [NT, 1], f32, kind="Internal").ap()
    ad2_d = nc.dram_tensor("ad2", [NT, 1], f32, kind="Internal").ap()

    # ---------------- phase A: gather tables + a_d arrays (replicated) -------
    with tc.tile_pool(name="pa", bufs=2) as pa, \
         tc.tile_pool(name="pa1", bufs=1) as pa1, \
         tc.tile_pool(name="pap", bufs=3, space="PSUM") as pap, \
         tc.tile_pool(name="pad", bufs=2, space="PSUM") as pad:
        wf = pa1.tile([P, 192], f16)
        nc.sync.dma_start(out=wf[:], in_=ins["wfull"][:])
        adw = pa1.tile([P, 2], f16)
        nc.sync.dma_start(out=adw[:], in_=ins["adw"][:])
        t_views = [t.rearrange("(g p) c -> p g c", p=P) for t in (t1_d, t2_d)]
        ad_views = [a.rearrange("(g p) c -> g (p c)", p=P) for a in (ad1_d, ad2_d)]
        for piece in range(NCORES):
            xt = pa.tile([P, NPC], f16, tag="xt")
            nc.sync.dma_start(
                out=xt[:], in_=ins["xT"][:, piece * NPC:(piece + 1) * NPC])
            # a_d arrays: [2, 512] psum per 4-tile group, stored straight out
            for j0 in range(0, CPC, ADG):
                r = min(ADG, CPC - j0)
                g0 = piece * CPC + j0
                psa = pad.tile([2, ADG * P], f32, tag="pad")
                nc.tensor.matmul(out=psa[:, :r * P], lhsT=adw[:],
                                 rhs=xt[:, j0 * P:(j0 + r) * P],
                                 start=True, stop=True)
                adsb = pa.tile([2, ADG * P], f32, tag="adsb")
                nc.vector.tensor_copy(out=adsb[:, :r * P], in_=psa[:, :r * P])
                for ti, adv in enumerate(ad_views):
                    nc.sync.dma_start(
                        out=adv[g0:g0 + r, :].rearrange("g c -> (g c)")[None, :],
                        in_=adsb[ti:ti + 1, :r * P])
            # gather tables
            for j0 in range(0, CPC, RND):
                r = min(RND, CPC - j0)
                g0 = piece * CPC + j0
                ps = pap.tile([P, 192 * RND], f32, tag="pap")
                for q in range(r):
                    nc.tensor.matmul(
                        out=ps[:, 192 * q:192 * (q + 1)],
                        lhsT=xt[:, (j0 + q) * P:(j0 + q + 1) * P],
                        rhs=wf[:], start=True, stop=True)
                psv = ps[:].rearrange("p (q c) -> p q c", q=RND)
                for ti, xw0 in ((0, 0), (1, 64)):
                    trow = pa.tile([P, RND * P], f16, tag=f"t{ti}r")
                    tv = trow[:].rearrange("p (q c) -> p q c", q=RND)
                    nc.vector.tensor_copy(
                        out=tv[:, :r, :], in_=psv[:, :r, xw0:xw0 + P])
                    if ti == 0:
                        nc.vector.tensor_copy(
                            out=tv[:, :r, 64:66], in_=psv[:, :r, 128:130])
                    nc.sync.dma_start(
                        out=t_views[ti][:, g0:g0 + r, :], in_=tv[:, :r, :])

    # ---------------- phase B: edge aggregation ----------------
    with tc.tile_pool(name="pre", bufs=1) as pre, \
         tc.tile_pool(name="pb", bufs=3) as pb, \
         tc.tile_pool(name="pg", bufs=2) as pg, \
         tc.tile_pool(name="pbp", bufs=4, space="PSUM") as pbp:
        iota = pre.tile([P, P], f16)
        nc.sync.dma_start(out=iota[:], in_=ins["iota"][:])
        bcomb = pre.tile([P, DOUT], f32)
        nc.sync.dma_start(out=bcomb[:], in_=ins["bcomb"][:])

        spaces = []   # (cv, bank, kbu, bo, maxw, table_ap, scol)
        for cv, tab, scol, kbua, kbub in (
                ("1", t1_d, SCOL[0], kbu1a, kbu1b),
                ("2", t2_d, SCOL[1], kbu2a, kbu2b)):
            for bn, kbu in (("a", kbua), ("b", kbub)):
                bo = np.zeros(CPC + 1, np.int64)
                bo[1:] = np.cumsum(kbu)
                maxw = max(int(bo[min(s + SCN, CPC)] - bo[s])
                           for s in range(0, CPC, SCN))
                tap = tab if (bn == "a" or NT <= BANK) else tab[BANK:, :]
                spaces.append(dict(cv=cv, bn=bn, kbu=kbu, bo=bo, maxw=maxw,
                                   tab=tap, scol=scol))
        kbmax = max(max(sp["kbu"]) for sp in spaces)
        dls = {}
        for sp in spaces:
            keyn = sp["cv"] + sp["bn"]
            kt = max(sum(sp["kbu"]), 1)
            t = pre.tile([P, kt], f16, tag="dl" + keyn)
            nc.sync.dma_start(out=t[:], in_=ins["dl" + keyn][:])
            dls[keyn] = t

        ad_tabs = {"1": ad1_d.rearrange("(g p) c -> g (p c)", p=P),
                   "2": ad2_d.rearrange("(g p) c -> g (p c)", p=P)}
        adix = pre.tile([P, CPC * 8], i16)
        nc.sync.dma_start(out=adix[:], in_=ins["adix"][:])

        if VARIANT == "phasea":
            for c in range(CPC):
                nc.sync.dma_start(out=out_d[c * P:(c + 1) * P, :],
                                  in_=bcomb[:])
            return

        for sc in range(0, CPC, SCN):
            scr = min(SCN, CPC - sc)
            sc_g = {}
            adbcs = {}
            for cv in ("1", "2"):
                t = pg.tile([P, SCN, P], f32, tag="adbc" + cv)
                nc.gpsimd.dma_gather(
                    out_ap=t[:, :scr, :], in_ap=ad_tabs[cv],
                    idxs_ap=adix[:, sc * 8:(sc + scr) * 8],
                    num_idxs=scr * P, num_idxs_reg=scr * P, elem_size=P)
                adbcs[cv] = t
            for sp in spaces:
                j0, j1 = int(sp["bo"][sc]), int(sp["bo"][sc + scr])
                jw = j1 - j0
                keyn = sp["cv"] + sp["bn"]
                if jw == 0:
                    sc_g[keyn] = (None, j0)
                    continue
                ixt = pg.tile([P, max(sp["maxw"], 1) * 8], i16, tag="ix" + keyn)
                nc.sync.dma_start(
                    out=ixt[:, :jw * 8],
                    in_=ins["ix" + keyn][:, j0 * 8:j1 * 8])
                gt = pg.tile([P, max(sp["maxw"], 1), P], f16, tag="gt" + keyn)
                for g0 in range(0, jw, GMAX):
                    gw = min(GMAX, jw - g0)
                    nc.gpsimd.dma_gather(
                        out_ap=gt[:, g0:g0 + gw, :], in_ap=sp["tab"],
                        idxs_ap=ixt[:, g0 * 8:(g0 + gw) * 8], num_idxs=gw * P,
                        num_idxs_reg=gw * P, elem_size=P)
                sc_g[keyn] = (gt, j0)

            if VARIANT == "gathers":
                for c in range(sc, sc + scr):
                    nc.sync.dma_start(out=out_d[c * P:(c + 1) * P, :],
                                      in_=bcomb[:])
                continue
            for c in range(sc, sc + scr):
                o1 = None
                psums = {}
                for cv in ("1", "2"):
                    ps = pbp.tile([P, 65], f32, tag="ps" + cv)
                    psums[cv] = ps
                    started = False
                    cvsp = [sp for sp in spaces if sp["cv"] == cv]
                    last_sp = [sp for sp in cvsp if int(sp["kbu"][c]) > 0][-1]
                    for sp in cvsp:
                        kb = int(sp["kbu"][c])
                        if kb == 0:
                            continue
                        keyn = cv + sp["bn"]
                        gt, j0 = sc_g[keyn]
                        cj = int(sp["bo"][c]) - j0
                        gts = gt[:, cj:cj + kb, :]
                        # one-hot
                        oh = pb.tile([P, kbmax, P], f16, tag="oh")
                        dlv = dls[keyn][:, int(sp["bo"][c]):
                                        int(sp["bo"][c]) + kb]
                        i1 = iota[:].unsqueeze(1)
                        nc.vector.tensor_tensor(
                            out=oh[:, :kb, :],
                            in0=dlv.unsqueeze(-1).to_broadcast([P, kb, P]),
                            in1=bass.AP(i1.tensor, i1.offset,
                                        [i1.ap[0], [0, kb], i1.ap[2]]),
                            op=mybir.AluOpType.is_equal)
                        # expand a_d through the one-hot: mul + reduce
                        ohw = pb.tile([P, kbmax, P], f16, tag="ohw")
                        ab = adbcs[cv][:, c - sc, :].unsqueeze(1)
                        nc.vector.tensor_tensor(
                            out=ohw[:, :kb, :], in0=oh[:, :kb, :],
                            in1=bass.AP(ab.tensor, ab.offset,
                                        [ab.ap[0], [0, kb], ab.ap[2]]),
                            op=mybir.AluOpType.mult)
                        ade = pb.tile([P, kbmax], f32, tag="ade")
                        nc.vector.tensor_reduce(
                            out=ade[:, :kb], in_=ohw[:, :kb, :],
                            axis=mybir.AxisListType.X, op=mybir.AluOpType.add)
                        # scores
                        u = pb.tile([P, kbmax], f32, tag="u")
                        nc.vector.tensor_tensor(
                            out=u[:, :kb], in0=gts[:, :, sp["scol"]],
                            in1=ade[:, :kb], op=mybir.AluOpType.add)
                        u2 = pb.tile([P, kbmax], f32, tag="u2")
                        nc.vector.tensor_scalar_mul(
                            u2[:, :kb], u[:, :kb], NEG_SLOPE)
                        nc.vector.tensor_tensor(
                            out=u2[:, :kb], in0=u[:, :kb], in1=u2[:, :kb],
                            op=mybir.AluOpType.max)
                        w = pb.tile([P, kbmax], f16, tag="w")
                        nc.scalar.activation(
                            out=w[:, :kb], in_=u2[:, :kb],
                            func=mybir.ActivationFunctionType.Exp)
                        # messages [w*xw | w]
                        msg = pb.tile([P, kbmax, 65], f16, tag="msg")
                        nc.vector.tensor_tensor(
                            out=msg[:, :kb, 0:64], in0=gts[:, :, 0:64],
                            in1=w[:, :kb].unsqueeze(-1).to_broadcast(
                                [P, kb, 64]),
                            op=mybir.AluOpType.mult)
                        nc.vector.tensor_copy(
                            out=msg[:, :kb, 64], in_=w[:, :kb])
                        for j in range(kb):
                            nc.tensor.matmul(
                                out=ps[:], lhsT=oh[:, j, :], rhs=msg[:, j, :],
                                start=not started,
                                stop=(sp is last_sp and j == kb - 1))
                            started = True
                # finalize
                for cv in ("1", "2"):
                    ps = psums[cv]
                    den = pb.tile([P, 1], f32, tag="den")
                    nc.vector.tensor_scalar_max(den[:], ps[:, 64:65], 1e-30)
                    rec = pb.tile([P, 1], f32, tag="rec")
                    nc.vector.reciprocal(out=rec[:], in_=den[:])
                    rec2 = pb.tile([P, 1], f32, tag="rec2")
                    nc.vector.tensor_scalar_mul(
                        rec2[:], rec[:], (1.0 - ALPHA) if cv == "1" else ALPHA)
                    o = pb.tile([P, DOUT], f32, tag="o" + cv)
                    nc.vector.tensor_scalar(
                        out=o[:], in0=ps[:, 0:64], scalar1=rec2[:],
                        scalar2=None, op0=mybir.AluOpType.mult)
                    if cv == "1":
                        o1 = o
                ofin = pb.tile([P, DOUT], f32, tag="ofin")
                nc.vector.tensor_tensor(
                    out=ofin[:], in0=o1[:], in1=o[:], op=mybir.AluOpType.add)
                nc.vector.tensor_tensor(
                    out=ofin[:], in0=ofin[:], in1=bcomb[:],
                    op=mybir.AluOpType.add)
                nc.sync.dma_start(out=out_d[c * P:(c + 1) * P, :], in_=ofin[:])


def _build(kbus):
    nc = bacc.Bacc("TRN2", target_bir_lowering=False, debug=False,
                   num_devices=NCORES)
    kbu1a, kbu1b, kbu2a, kbu2b = kbus
    ins = {
        "xT": nc.dram_tensor("xT", [DIN, NT], f16, kind="ExternalInput").ap(),
        "adix": nc.dram_tensor("adix", [P, CPC * 8], i16,
                               kind="ExternalInput").ap(),
        "wfull": nc.dram_tensor("wfull", [DIN, 192], f16,
                                kind="ExternalInput").ap(),
        "adw": nc.dram_tensor("adw", [DIN, 2], f16,
                              kind="ExternalInput").ap(),
        "iota": nc.dram_tensor("iota", [P, P], f16, kind="ExternalInput").ap(),
        "bcomb": nc.dram_tensor("bcomb", [P, DOUT], f32,
                                kind="ExternalInput").ap(),
    }
    for nm, kbu in (("1a", kbu1a), ("1b", kbu1b), ("2a", kbu2a),
                    ("2b", kbu2b)):
        kt = max(sum(kbu), 1)
        ins["ix" + nm] = nc.dram_tensor(
            "ix" + nm, [P, kt * 8], i16, kind="ExternalInput").ap()
        ins["dl" + nm] = nc.dram_tensor(
            "dl" + nm, [P, kt], f16, kind="ExternalInput").ap()
    outs = {"out": nc.dram_tensor("out", [NPC, DOUT], f32,
                                  kind="ExternalOutput").ap()}
    with tile.TileContext(nc) as tc:
        _emit(tc, outs, ins, kbus)
    nc.compile()
    return nc


# ------------------------------------------------------------------- entry
def kernel(x, edge_index, W1, att_src1, att_dst1, b1,
           W2, att_src2, att_dst2, b2):
    common, per_core, kbus = _preprocess(
        np.asarray(x), np.asarray(edge_index),
        np.asarray(W1, np.float64), np.asarray(att_src1, np.float64),
        np.asarray(att_dst1, np.float64), np.asarray(b1, np.float32),
        np.asarray(W2, np.float64), np.asarray(att_src2, np.float64),
        np.asarray(att_dst2, np.float64), np.asarray(b2, np.float32))

    if kbus not in _CACHE:
        _CACHE[kbus] = _build(kbus)
    nc = _CACHE[kbus]

    in_maps = [dict(common, **pc) for pc in per_core]
    res = bass_utils.run_bass_kernel_spmd(
        nc, in_maps, core_ids=list(range(NCORES)))
    full = np.concatenate(
        [res.results[k]["out"] for k in range(NCORES)], axis=0)
    return np.ascontiguousarray(full[:N]).astype(np.float32)


# revision 25
# speedup vs baseline: 9.0032x; 7.3164x over previous
"""DirGATConv on 8 Trainium2 NeuronCores (Bass/Tile).

Problem: nn_DirGATConv  (N=50000 nodes, E=800000 edges, DIN=128, DOUT=64)
    out = 0.5 * GATConv(x, src->dst, W1) + 0.5 * GATConv(x, dst->src, W2)

Strategy (zero collectives):
  * Nodes are grouped into 128-row "chunks"; chunks are assigned to cores in
    contiguous ranges (49 chunk slots per core).  Conv1 groups edges by dst,
    conv2 by src; both groupings use the same node chunks, so each core owns
    all edges whose *group* node lies in its range and produces the final
    output rows for exactly its node range.  The host concatenates slices.
  * Phase A (replicated on every core): one fused matmul computes
    xw1|xw2|attention projections for all nodes; rows of the per-conv gather
    tables T1/T2 are [xw (64 f16) | a_src (f16) | ...] = 256 B, plus flat
    f32 a_dst arrays.
  * Phase B: per-edge source rows are fetched with the custom DMA gather
    (InstDMAGatherAnt, int16 indices, so the node space is split into two
    banks with separate slot spaces).  Per-edge dst-attention is expanded
    from a per-chunk broadcast tile through the one-hot matrix
    (mul + reduce).  Messages [w*xw | w] are accumulated per 128-node chunk
    on the PE via one-hot matmuls into PSUM [128, 65]; the denominator is
    column 64.  Softmax max-subtraction is skipped (|scores| <= ~3, exp is
    safe in fp32, and the quotient is unchanged).
"""

import math

import numpy as np

import concourse.bass as bass
import concourse.mybir as mybir
import concourse.tile as tile
from concourse import bacc, bass_utils
from concourse._compat import with_exitstack

# ---------------------------------------------------------------- constants
N = 50000
E = 800000
DIN = 128
DOUT = 64
ALPHA = 0.5
NEG_SLOPE = 0.2
NCORES = 8
P = 128

G_REAL = math.ceil(N / P)              # 391 real node chunks
CPC = math.ceil(G_REAL / NCORES)       # 49 chunk slots per core
G = NCORES * CPC                       # 392 padded chunk slots
NT = G * P                             # 50176 padded node count
NPC = CPC * P                          # 6272 nodes per core
BANK = 32768                           # int16 gather-index bank size
RND = 2                                # node tiles per PSUM round in phase A
ADG = 4                                # node tiles per a_d matmul group
SCN = 3                                # chunks per gather super-chunk
GMAX = 8                               # max 128-blocks per gather (1024-desc ring)
SCOL = (64, 66)                        # a_src column in T1 / T2 rows

f32 = mybir.dt.float32
f16 = mybir.dt.float16
i16 = mybir.dt.int16

VARIANT = "full"    # "full" | "gathers" | "phasea"  (perf-bisect variants)

_CACHE = {}


# ------------------------------------------------------------ host preprocess
def _edge_arrays(key, gidx):
    """Bin edges by group node ("key"), 128 nodes per chunk; within a chunk
    split edges by gather-node bank (<BANK or >=BANK) into two slot spaces.
    Slot i of a span maps to partition i%128, block i//128.  Returns per-bank
    (idx16 wrapped [NCORES,128,Ktot*8], dl [NCORES,128,Ktot]) plus per-bank
    block counts kbu[CPC] (shared across cores)."""
    order = np.lexsort((gidx, gidx >= BANK, key // P))
    key_s = key[order].astype(np.int64)
    gid_s = gidx[order].astype(np.int64)
    bank_s = (gid_s >= BANK).astype(np.int64)

    chunk = key_s // P
    # counts per (chunk, bank)
    cb = chunk * 2 + bank_s
    counts = np.bincount(cb, minlength=2 * G).reshape(G, 2)
    kb = -(-counts // P)                                  # [G, 2]
    kbA = np.maximum(kb[:, 0].reshape(NCORES, CPC).max(axis=0), 1)
    kbB = kb[:, 1].reshape(NCORES, CPC).max(axis=0)
    out = []
    start = np.zeros(2 * G + 1, np.int64)
    start[1:] = np.cumsum(counts.reshape(-1))
    within = np.arange(key_s.size, dtype=np.int64) - start[cb]
    core = chunk // CPC
    slot = chunk % CPC
    for b, kbu in ((0, kbA), (1, kbB)):
        BO = np.zeros(CPC + 1, np.int64)
        BO[1:] = np.cumsum(kbu)
        ktot = int(BO[-1])
        sel = bank_s == b
        s = BO[slot[sel]] * P + within[sel]
        idx = np.zeros((NCORES, max(ktot, 1) * P), np.int16)
        dl = np.full((NCORES, max(ktot, 1) * P), -1.0, np.float16)
        idx[core[sel], s] = (gid_s[sel] - b * BANK).astype(np.int16)
        dl[core[sel], s] = (key_s[sel] - chunk[sel] * P).astype(np.float16)
        # wrap int16 indices: position i -> [p=i%16, j=i//16], replicated x8
        idxw = idx.reshape(NCORES, max(ktot, 1) * 8, 16).transpose(0, 2, 1)
        idxw = np.ascontiguousarray(np.tile(idxw, (1, 8, 1)))
        dl2 = np.ascontiguousarray(
            dl.reshape(NCORES, max(ktot, 1), P).transpose(0, 2, 1))
        out.append((idxw, dl2, tuple(int(x) for x in kbu)))
    return out


def _preprocess(x, edge_index, W1, att_src1, att_dst1, b1,
                W2, att_src2, att_dst2, b2):
    src = np.asarray(edge_index[0], np.int64)
    dst = np.asarray(edge_index[1], np.int64)
    loops = np.arange(N, dtype=np.int64)
    all_src = np.concatenate([src, loops])
    all_dst = np.concatenate([dst, loops])

    c1 = _edge_arrays(all_dst, all_src)    # conv1: group by dst, gather src
    c2 = _edge_arrays(all_src, all_dst)    # conv2: group by src, gather dst

    xT = np.zeros((DIN, NT), np.float16)
    xT[:, :N] = np.asarray(x, np.float32).T.astype(np.float16)

    wfull = np.zeros((DIN, 192), np.float32)
    wfull[:, 0:64] = W1
    wfull[:, 64:128] = W2
    wfull[:, 128] = W1 @ att_src1
    wfull[:, 129] = W1 @ att_dst1
    wfull[:, 130] = W2 @ att_src2
    wfull[:, 131] = W2 @ att_dst2
    wfull = wfull.astype(np.float16)
    adw = np.stack([W1 @ att_dst1, W2 @ att_dst2], axis=1).astype(np.float16)

    iota = np.broadcast_to(np.arange(P, dtype=np.float16), (P, P)).copy()
    bcomb = np.broadcast_to(
        ((1.0 - ALPHA) * b1 + ALPHA * b2).astype(np.float32), (P, DOUT)).copy()

    common = dict(xT=xT, wfull=wfull, adw=adw, iota=iota, bcomb=bcomb)
    per_core = []
    for k in range(NCORES):
        d = {}
        for cv, banks in (("1", c1), ("2", c2)):
            for bn, (idxw, dl2, _kbu) in zip("ab", banks):
                d["ix" + cv + bn] = idxw[k]
                d["dl" + cv + bn] = dl2[k]
        # per-chunk a_d row ids (global chunk index), wrapped for dma_gather
        af = np.repeat(np.arange(k * CPC, (k + 1) * CPC), P).astype(np.int16)
        d["adix"] = np.tile(af.reshape(CPC * 8, 16).T, (8, 1)).astype(np.int16)
        per_core.append(d)
    kbus = tuple(banks[b][2] for banks in (c1, c2) for b in (0, 1))
    return common, per_core, kbus


# ------------------------------------------------------------- device program
@with_exitstack
def _emit(ctx, tc, outs, ins, kbus):
    nc = tc.nc
    out_d = outs["out"]
    kbu1a, kbu1b, kbu2a, kbu2b = kbus

    t1_d = nc.dram_tensor("T1_tab", [NT, P], f16, kind="Internal").ap()
    t2_d = nc.dram_tensor("T2_tab", [NT, P], f16, kind="Internal").ap()
    ad1_d = nc.dram_tensor("ad1", [NT, 1], f32, kind="Internal").ap()
    ad2_d = nc.dram_tensor("ad2", [NT, 1], f32, kind="Internal").ap()

    # ---------------- phase A: gather tables + a_d arrays (replicated) -------
    with tc.tile_pool(name="pa", bufs=2) as pa, \
         tc.tile_pool(name="pa1", bufs=1) as pa1, \
         tc.tile_pool(name="pap", bufs=3, space="PSUM") as pap, \
         tc.tile_pool(name="pad", bufs=2, space="PSUM") as pad:
        wf = pa1.tile([P, 192], f16)
        nc.sync.dma_start(out=wf[:], in_=ins["wfull"][:])
        adw = pa1.tile([P, 2], f16)
        nc.sync.dma_start(out=adw[:], in_=ins["adw"][:])
        t_views = [t.rearrange("(g p) c -> p g c", p=P) for t in (t1_d, t2_d)]
        ad_views = [a.rearrange("(g p) c -> g (p c)", p=P) for a in (ad1_d, ad2_d)]
        for piece in range(NCORES):
            xt = pa.tile([P, NPC], f16, tag="xt")
            nc.sync.dma_start(
                out=xt[:], in_=ins["xT"][:, piece * NPC:(piece + 1) * NPC])
            # a_d arrays: [2, 512] psum per 4-tile group, stored straight out
            for j0 in range(0, CPC, ADG):
                r = min(ADG, CPC - j0)
                g0 = piece * CPC + j0
                psa = pad.tile([2, ADG * P], f32, tag="pad")
                nc.tensor.matmul(out=psa[:, :r * P], lhsT=adw[:],
                                 rhs=xt[:, j0 * P:(j0 + r) * P],
                                 start=True, stop=True)
                adsb = pa.tile([2, ADG * P], f32, tag="adsb")
                nc.vector.tensor_copy(out=adsb[:, :r * P], in_=psa[:, :r * P])
                for ti, adv in enumerate(ad_views):
                    nc.sync.dma_start(
                        out=adv[g0:g0 + r, :].rearrange("g c -> (g c)")[None, :],
                        in_=adsb[ti:ti + 1, :r * P])
            # gather tables
            for j0 in range(0, CPC, RND):
                r = min(RND, CPC - j0)
                g0 = piece * CPC + j0
                ps = pap.tile([P, 192 * RND], f32, tag="pap")
                for q in range(r):
                    nc.tensor.matmul(
                        out=ps[:, 192 * q:192 * (q + 1)],
                        lhsT=xt[:, (j0 + q) * P:(j0 + q + 1) * P],
                        rhs=wf[:], start=True, stop=True)
                psv = ps[:].rearrange("p (q c) -> p q c", q=RND)
                for ti, xw0 in ((0, 0), (1, 64)):
                    trow = pa.tile([P, RND * P], f16, tag=f"t{ti}r")
                    tv = trow[:].rearrange("p (q c) -> p q c", q=RND)
                    nc.vector.tensor_copy(
                        out=tv[:, :r, :], in_=psv[:, :r, xw0:xw0 + P])
                    if ti == 0:
                        nc.vector.tensor_copy(
                            out=tv[:, :r, 64:66], in_=psv[:, :r, 128:130])
                    nc.sync.dma_start(
                        out=t_views[ti][:, g0:g0 + r, :], in_=tv[:, :r, :])

    # ---------------- phase B: edge aggregation ----------------
    with tc.tile_pool(name="pre", bufs=1) as pre, \
         tc.tile_pool(name="pb", bufs=3) as pb, \
         tc.tile_pool(name="pg", bufs=2) as pg, \
         tc.tile_pool(name="pbp", bufs=4, space="PSUM") as pbp:
        iota = pre.tile([P, P], f16)
        nc.sync.dma_start(out=iota[:], in_=ins["iota"][:])
        bcomb = pre.tile([P, DOUT], f32)
        nc.sync.dma_start(out=bcomb[:], in_=ins["bcomb"][:])

        spaces = []   # (cv, bank, kbu, bo, maxw, table_ap, scol)
        for cv, tab, scol, kbua, kbub in (
                ("1", t1_d, SCOL[0], kbu1a, kbu1b),
                ("2", t2_d, SCOL[1], kbu2a, kbu2b)):
            for bn, kbu in (("a", kbua), ("b", kbub)):
                bo = np.zeros(CPC + 1, np.int64)
                bo[1:] = np.cumsum(kbu)
                maxw = max(int(bo[min(s + SCN, CPC)] - bo[s])
                           for s in range(0, CPC, SCN))
                tap = tab if (bn == "a" or NT <= BANK) else tab[BANK:, :]
                spaces.append(dict(cv=cv, bn=bn, kbu=kbu, bo=bo, maxw=maxw,
                                   tab=tap, scol=scol))
        kbmax = max(max(sp["kbu"]) for sp in spaces)
        dls = {}
        for sp in spaces:
            keyn = sp["cv"] + sp["bn"]
            kt = max(sum(sp["kbu"]), 1)
            t = pre.tile([P, kt], f16, tag="dl" + keyn)
            nc.sync.dma_start(out=t[:], in_=ins["dl" + keyn][:])
            dls[keyn] = t

        ad_tabs = {"1": ad1_d.rearrange("(g p) c -> g (p c)", p=P),
                   "2": ad2_d.rearrange("(g p) c -> g (p c)", p=P)}
        adix = pre.tile([P, CPC * 8], i16)
        nc.sync.dma_start(out=adix[:], in_=ins["adix"][:])

        if VARIANT == "phasea":
            for c in range(CPC):
                nc.sync.dma_start(out=out_d[c * P:(c + 1) * P, :],
                                  in_=bcomb[:])
            return

        for sc in range(0, CPC, SCN):
            scr = min(SCN, CPC - sc)
            sc_g = {}
            adbcs = {}
            for cv in ("1", "2"):
                t = pg.tile([P, SCN, P], f32, tag="adbc" + cv)
                nc.gpsimd.dma_gather(
                    out_ap=t[:, :scr, :], in_ap=ad_tabs[cv],
                    idxs_ap=adix[:, sc * 8:(sc + scr) * 8],
                    num_idxs=scr * P, num_idxs_reg=scr * P, elem_size=P)
                adbcs[cv] = t
            for sp in spaces:
                j0, j1 = int(sp["bo"][sc]), int(sp["bo"][sc + scr])
                jw = j1 - j0
                keyn = sp["cv"] + sp["bn"]
                if jw == 0:
                    sc_g[keyn] = (None, j0)
                    continue
                ixt = pg.tile([P, max(sp["maxw"], 1) * 8], i16, tag="ix" + keyn)
                nc.sync.dma_start(
                    out=ixt[:, :jw * 8],
                    in_=ins["ix" + keyn][:, j0 * 8:j1 * 8])
                gt = pg.tile([P, max(sp["maxw"], 1), P], f16, tag="gt" + keyn)
                for g0 in range(0, jw, GMAX):
                    gw = min(GMAX, jw - g0)
                    nc.gpsimd.dma_gather(
                        out_ap=gt[:, g0:g0 + gw, :], in_ap=sp["tab"],
                        idxs_ap=ixt[:, g0 * 8:(g0 + gw) * 8], num_idxs=gw * P,
                        num_idxs_reg=gw * P, elem_size=P)
                sc_g[keyn] = (gt, j0)

            if VARIANT == "gathers":
                for c in range(sc, sc + scr):
                    nc.sync.dma_start(out=out_d[c * P:(c + 1) * P, :],
                                      in_=bcomb[:])
                continue
            for c in range(sc, sc + scr):
                o1 = None
                psums = {}
                for cv in ("1", "2"):
                    ps = pbp.tile([P, 65], f32, tag="ps" + cv)
                    psums[cv] = ps
                    started = False
                    cvsp = [sp for sp in spaces if sp["cv"] == cv]
                    last_sp = [sp for sp in cvsp if int(sp["kbu"][c]) > 0][-1]
                    for sp in cvsp:
                        kb = int(sp["kbu"][c])
                        if kb == 0:
                            continue
                        keyn = cv + sp["bn"]
                        gt, j0 = sc_g[keyn]
                        cj = int(sp["bo"][c]) - j0
                        gts = gt[:, cj:cj + kb, :]
                        # one-hot
                        oh = pb.tile([P, kbmax, P], f16, tag="oh")
                        dlv = dls[keyn][:, int(sp["bo"][c]):
                                        int(sp["bo"][c]) + kb]
                        i1 = iota[:].unsqueeze(1)
                        nc.vector.tensor_tensor(
                            out=oh[:, :kb, :],
                            in0=dlv.unsqueeze(-1).to_broadcast([P, kb, P]),
                            in1=bass.AP(i1.tensor, i1.offset,
                                        [i1.ap[0], [0, kb], i1.ap[2]]),
                            op=mybir.AluOpType.is_equal)
                        # expand a_d through the one-hot: mul + reduce
                        ohw = pb.tile([P, kbmax, P], f16, tag="ohw")
                        ab = adbcs[cv][:, c - sc, :].unsqueeze(1)
                        nc.vector.tensor_tensor(
                            out=ohw[:, :kb, :], in0=oh[:, :kb, :],
                            in1=bass.AP(ab.tensor, ab.offset,
                                        [ab.ap[0], [0, kb], ab.ap[2]]),
                            op=mybir.AluOpType.mult)
                        ade = pb.tile([P, kbmax], f32, tag="ade")
                        nc.vector.tensor_reduce(
                            out=ade[:, :kb], in_=ohw[:, :kb, :],
                            axis=mybir.AxisListType.X, op=mybir.AluOpType.add)
                        # scores
                        u = pb.tile([P, kbmax], f32, tag="u")
                        nc.vector.tensor_tensor(
                            out=u[:, :kb], in0=gts[:, :, sp["scol"]],
                            in1=ade[:, :kb], op=mybir.AluOpType.add)
                        u2 = pb.tile([P, kbmax], f32, tag="u2")
                        nc.vector.tensor_scalar_mul(
                            u2[:, :kb], u[:, :kb], NEG_SLOPE)
                        nc.vector.tensor_tensor(
                            out=u2[:, :kb], in0=u[:, :kb], in1=u2[:, :kb],
                            op=mybir.AluOpType.max)
                        w = pb.tile([P, kbmax], f16, tag="w")
                        nc.scalar.activation(
                            out=w[:, :kb], in_=u2[:, :kb],
                            func=mybir.ActivationFunctionType.Exp)
                        # messages [w*xw | w]
                        msg = pb.tile([P, kbmax, 65], f16, tag="msg")
                        nc.vector.tensor_tensor(
                            out=msg[:, :kb, 0:64], in0=gts[:, :, 0:64],
                            in1=w[:, :kb].unsqueeze(-1).to_broadcast(
                                [P, kb, 64]),
                            op=mybir.AluOpType.mult)
                        nc.vector.tensor_copy(
                            out=msg[:, :kb, 64], in_=w[:, :kb])
                        for j in range(kb):
                            nc.tensor.matmul(
                                out=ps[:], lhsT=oh[:, j, :], rhs=msg[:, j, :],
                                start=not started,
                                stop=(sp is last_sp and j == kb - 1))
                            started = True
                # finalize
                for cv in ("1", "2"):
                    ps = psums[cv]
                    den = pb.tile([P, 1], f32, tag="den")
                    nc.vector.tensor_scalar_max(den[:], ps[:, 64:65], 1e-30)
                    rec = pb.tile([P, 1], f32, tag="rec")
                    nc.vector.reciprocal(out=rec[:], in_=den[:])
                    rec2 = pb.tile([P, 1], f32, tag="rec2")
                    nc.vector.tensor_scalar_mul(
                        rec2[:], rec[:], (1.0 - ALPHA) if cv == "1" else ALPHA)
                    o = pb.tile([P, DOUT], f32, tag="o" + cv)
                    nc.vector.tensor_scalar(
                        out=o[:], in0=ps[:, 0:64], scalar1=rec2[:],
                        scalar2=None, op0=mybir.AluOpType.mult)
                    if cv == "1":
                        o1 = o
                ofin = pb.tile([P, DOUT], f32, tag="ofin")
                nc.vector.tensor_tensor(
                    out=ofin[:], in0=o1[:], in1=o[:], op=mybir.AluOpType.add)
                nc.vector.tensor_tensor(
                    out=ofin[:], in0=ofin[:], in1=bcomb[:],
                    op=mybir.AluOpType.add)
                nc.sync.dma_start(out=out_d[c * P:(c + 1) * P, :], in_=ofin[:])


def _build(kbus):
    nc = bacc.Bacc("TRN2", target_bir_lowering=False, debug=False,
                   num_devices=NCORES)
    kbu1a, kbu1b, kbu2a, kbu2b = kbus
    ins = {
        "xT": nc.dram_tensor("xT", [DIN, NT], f16, kind="ExternalInput").ap(),
        "adix": nc.dram_tensor("adix", [P, CPC * 8], i16,
                               kind="ExternalInput").ap(),
        "wfull": nc.dram_tensor("wfull", [DIN, 192], f16,
                                kind="ExternalInput").ap(),
        "adw": nc.dram_tensor("adw", [DIN, 2], f16,
                              kind="ExternalInput").ap(),
        "iota": nc.dram_tensor("iota", [P, P], f16, kind="ExternalInput").ap(),
        "bcomb": nc.dram_tensor("bcomb", [P, DOUT], f32,
                                kind="ExternalInput").ap(),
    }
    for nm, kbu in (("1a", kbu1a), ("1b", kbu1b), ("2a", kbu2a),
                    ("2b", kbu2b)):
        kt = max(sum(kbu), 1)
        ins["ix" + nm] = nc.dram_tensor(
            "ix" + nm, [P, kt * 8], i16, kind="ExternalInput").ap()
        ins["dl" + nm] = nc.dram_tensor(
            "dl" + nm, [P, kt], f16, kind="ExternalInput").ap()
    outs = {"out": nc.dram_tensor("out", [NPC, DOUT], f32,
                                  kind="ExternalOutput").ap()}
    with tile.TileContext(nc) as tc:
        _emit(tc, outs, ins, kbus)
    nc.compile()
    return nc


# ------------------------------------------------------------------- entry
def kernel(x, edge_index, W1, att_src1, att_dst1, b1,
           W2, att_src2, att_dst2, b2):
    common, per_core, kbus = _preprocess(
        np.asarray(x), np.asarray(edge_index),
        np.asarray(W1, np.float64), np.asarray(att_src1, np.float64),
        np.asarray(att_dst1, np.float64), np.asarray(b1, np.float32),
        np.asarray(W2, np.float64), np.asarray(att_src2, np.float64),
        np.asarray(att_dst2, np.float64), np.asarray(b2, np.float32))

    if kbus not in _CACHE:
        _CACHE[kbus] = _build(kbus)
    nc = _CACHE[kbus]

    in_maps = [dict(common, **pc) for pc in per_core]
    res = bass_utils.run_bass_kernel_spmd(
        nc, in_maps, core_ids=list(range(NCORES)))
    full = np.concatenate(
        [res.results[k]["out"] for k in range(NCORES)], axis=0)
    return np.ascontiguousarray(full[:N]).astype(np.float32)
